# revision 20
# baseline (speedup 1.0000x reference)
"""Trainium2 Bass kernel for nn_Attention_83081847374268 (sparse sliding-window GQA).

Sharding: 8 cores = batch (2, data parallel) x kv-head (4, tensor parallel).
Each core computes, for its (b, kh): q/k/v projections (2 q heads, 1 kv head),
QK-RMSNorm + RoPE, banded sliding-window attention, and a partial output
projection against its 512-row slice of wout.  The host sums the 4 fp16
partials per batch (the TP reduction) and stacks the batches.

Device dataflow (per core), all heavy matmuls in bf16 (PSUM accum f32):
  stage A: stream x column-chunks (bf16, host-pretiled so every DMA is a
           contiguous >=4KB run per partition); projections directly in
           transposed layout (qT/kT [head_dim, T] bf16); RMSNorm via
           ones-matmul variance + PE-broadcast rstd; RoPE fused with the
           rstd multiply on DVE.  The PE-only norm-tail instructions of
           each unit are deferred behind the next unit's main matmuls so
           the PE never waits on the ACT sqrt/square chain.
           v PE-transposed into a persistent SBUF tile (no DRAM bounce).
  stage B: per 128-query tile (one tci behind stage A): S = qT.T @ kT over
           the host-derived non-empty key chunks only; boundary chunks get
           an additive mask tile (DVE, in PSUM); exp reads S straight from
           PSUM with the deferred 1/std as a per-row ACT scale and fused
           row-sum; P normalized on DVE in bf16, PE-transposed; per-tile PV
           accumulates encoded^T; the out-projection of tile t-1 is
           interleaved behind tile t's S/softmax so the PE stays dense and
           the kernel tail is one tile deep, not one 2-tile group deep.

DMA: x/wk + outputs ride the sync HWDGE queue; all other weights/tables
prefetch on the gpsimd SWDGE queue so descriptor generation and transfers
overlap from t=0.
"""
import sys

sys.path.insert(0, "/opt/trn_rl_repo")

import numpy as np
import ml_dtypes

import concourse.bacc as bacc
import concourse.mybir as mybir
from concourse.bass_utils import run_bass_kernel_spmd
from concourse.tile import TileContext
from concourse.alu_op_type import AluOpType

F32 = mybir.dt.float32
F32R = mybir.dt.float32r
BF16 = mybir.dt.bfloat16
F16 = mybir.dt.float16
ACTF = mybir.ActivationFunctionType

B, T, WIDTH = 2, 2048, 2048
NUM_HEADS, NUM_KV_HEADS, HEAD_DIM = 8, 4, 256
GROUPS = NUM_HEADS // NUM_KV_HEADS  # 2 q heads per kv head (= per core)
WINDOW = 512
ROPE_BASE = 10000.0
ALPHA = HEAD_DIM ** -0.5

NT = T // 128           # 16 query tiles
TCH = 512               # stage-A t-chunk width
NTCH = T // TCH         # 4
NW = WIDTH // 128       # 16 contraction chunks

_prog_cache = {}


def _round_up(x, m):
    return (x + m - 1) // m * m


def _geometry(positions, attn_mask):
    """Per-query-tile key windows from the actual mask/positions data."""
    pos = np.asarray(positions)
    am = np.asarray(attn_mask)
    pd = pos[:, :, None].astype(np.int64) - pos[:, None, :].astype(np.int64)
    valid = am & (np.abs(pd) < WINDOW)  # [B, T, T] bool
    assert valid.any(axis=2).all(), "a query row with no valid key is unsupported"
    js = []
    wmax = 0
    for it in range(NT):
        cols = valid[:, it * 128:(it + 1) * 128, :].any(axis=(0, 1))
        idx = np.nonzero(cols)[0]
        j_lo, j_hi = int(idx[0]), int(idx[-1]) + 1
        j0 = (j_lo // 128) * 128
        wmax = max(wmax, j_hi - j0)
        js.append(j0)
    Wb = max(256, _round_up(wmax, 128))
    Wb = min(Wb, T)
    js = tuple(max(0, min(j, T - Wb)) for j in js)
    return valid, Wb, js


def _pieces(w):
    """Split w (multiple of 128) into moving-dim pieces of <=512 (PSUM bank)."""
    out = []
    rem = w
    while rem > 512:
        take = 512 if rem - 512 >= 256 or rem == 1024 else rem - 256
        out.append(take)
        rem -= take
    if rem:
        out.append(rem)
    return out


def _classify(valid, js, Wb):
    """Per (query tile, window chunk): 'full' / 'empty' / ('mask', slot)."""
    NJ = Wb // 128
    pats = {}
    plan = []
    for it in range(NT):
        base = js[it] // 128
        row = []
        for i in range(NJ):
            jc = base + i
            sl = valid[:, it * 128:(it + 1) * 128, jc * 128:(jc + 1) * 128]
            if sl.all():
                row.append(("full", None))
            elif not sl.any():
                row.append(("empty", None))
            else:
                key = sl.tobytes()
                if key not in pats:
                    pats[key] = (len(pats), sl.copy())
                row.append(("mask", pats[key][0]))
        plan.append(tuple(row))
    patterns = [sl for _, sl in sorted(pats.values(), key=lambda v: v[0])]
    return tuple(plan), patterns


def _runs(row):
    """Contiguous runs of non-empty chunks: list of (c0, c1)."""
    runs = []
    c = 0
    NJ = len(row)
    while c < NJ:
        if row[c][0] == "empty":
            c += 1
            continue
        c0 = c
        while c < NJ and row[c][0] != "empty":
            c += 1
        runs.append((c0, c))
    return runs


def _rope_tables(pos_b, scale):
    """cos/sin tables in [head_dim/2, T] (transposed) layout, gain folded in."""
    d = np.arange(HEAD_DIM // 2, dtype=np.float32)
    timescale = (ROPE_BASE ** (2.0 / HEAD_DIM * d)).astype(np.float32)
    rad = pos_b.astype(np.float32)[None, :] / timescale[:, None]  # [128, T]
    cos, sin = np.cos(rad).astype(np.float32), np.sin(rad).astype(np.float32)
    g1 = (1.0 + scale[:HEAD_DIM // 2]).astype(np.float32)[:, None]
    g2 = (1.0 + scale[HEAD_DIM // 2:]).astype(np.float32)[:, None]
    # o1 = a1*C1 - a2*S2 ; o2 = a2*C2 + a1*S1
    return (cos * g1, sin * g1, cos * g2, sin * g2)  # C1, S1, C2, S2


def _build(Wb, js, plan, n_pat, shared_tables):
    nc = bacc.Bacc("TRN2", target_bir_lowering=False, debug=False, num_devices=8)

    def din(name, shape, dt):
        return nc.dram_tensor(name, shape, dt, kind="ExternalInput").ap()

    xt4 = din("xt4", [128, NTCH, NW, TCH], BF16)
    wq4 = din("wq4", [128, NW, 512], BF16)
    wk4 = din("wk4", [128, NW, 256], BF16)
    wv4 = din("wv4", [128, NW, 256], BF16)
    wout = din("wout", [512, T], BF16)
    ident_d = din("ident", [128, 128], BF16)
    ones1_d = din("ones1", [1, 128], F32R)    # K=1 broadcast lhsT
    onesc_d = din("onesc", [128, 1], F32R)    # partition-sum lhsT
    masks_d = din("masks", [128, max(n_pat, 1) * 128], BF16)
    tab_names = ["ct", "st"] if shared_tables else [
        "cq1", "sq1", "cq2", "sq2", "ck1", "sk1", "ck2", "sk2"]
    tabs = {n: din(n, [128, T], BF16) for n in tab_names}
    yp = nc.dram_tensor("yp", [T, T], F16, kind="ExternalOutput").ap()
    warm_out = nc.dram_tensor("warm_out", [1, 8], F32, kind="ExternalOutput").ap()

    NJ = Wb // 128  # window chunks per tile
    # per-tile live (non-empty) absolute key chunks
    tile_chunks = []
    for it in range(NT):
        base = js[it] // 128
        tile_chunks.append([base + i for i in range(NJ)
                            if plan[it][i][0] != "empty"])

    with TileContext(nc) as tc:
        with (
            tc.tile_pool(name="persist", bufs=1) as pp,
            tc.tile_pool(name="qk_store", bufs=1) as qkp,
        ):
            ident = pp.tile([128, 128], BF16)
            nc.sync.dma_start(out=ident[:], in_=ident_d[:])
            ones1 = pp.tile([1, 128], F32R)
            nc.sync.dma_start(out=ones1[:], in_=ones1_d[:])
            onesc = pp.tile([128, 1], F32R)
            nc.sync.dma_start(out=onesc[:], in_=onesc_d[:])
            epsb = pp.tile([1, 1], F32)
            nc.any.memset(epsb[:], 1e-6)
            epsbq = pp.tile([1, 1], F32)
            nc.any.memset(epsbq[:], HEAD_DIM * 1e-6)
            ones_f = pp.tile([1, 1], F32)
            nc.any.memset(ones_f[:], 1.0)
            rstdq_c = [pp.tile([128, NT], F32, tag=f"rstdq{hh}", name=f"rstdq{hh}")
                       for hh in range(2)]
            qT = [qkp.tile([128, T], BF16, tag=f"qT{c}", name=f"qT{c}") for c in range(4)]
            kT = [qkp.tile([128, T], BF16, tag=f"kT{c}", name=f"kT{c}") for c in range(2)]
            # v resident in SBUF, natural [keys, head_dim] layout per 128-chunk
            vt = qkp.tile([128, NT * 256], BF16, tag="vt", name="vt")

            ps_shared = tc.tile_pool(name="ps_shared", bufs=2, space="PSUM")
            psA = psA1 = psS = psT = psE = ps_shared.__enter__()
            with (
                tc.tile_pool(name="wpool", bufs=1) as wp,
                tc.tile_pool(name="xpool", bufs=2) as xp,
                tc.tile_pool(name="tabpool", bufs=1) as tp,
                tc.tile_pool(name="sa", bufs=1) as sa,
                tc.tile_pool(name="encp", bufs=1) as encp,
                tc.tile_pool(name="woutp", bufs=1) as woutp,
                tc.tile_pool(name="sb", bufs=2) as sbp,
                tc.tile_pool(name="ptp", bufs=2) as ptp,
                tc.tile_pool(name="outp", bufs=3) as outp,
            ):
                wq_t = wp.tile([128, NW * 512], BF16)
                wk_t = wp.tile([128, NW * 256], BF16)
                wv_t = wp.tile([128, NW * 256], BF16)
                wq_v = wq_t[:].rearrange("p (c m) -> p c m", m=512)
                wk_v = wk_t[:].rearrange("p (c m) -> p c m", m=256)
                wv_v = wv_t[:].rearrange("p (c m) -> p c m", m=256)

                def load_xts(tci):
                    xts = xp.tile([128, NW * TCH], BF16, tag="xts", name=f"xts{tci}")
                    xv = xts[:].rearrange("p (c t) -> p c t", t=TCH)
                    for q4 in range(4):
                        nc.sync.dma_start(out=xv[:, q4 * 4:(q4 + 1) * 4],
                                          in_=xt4[:, tci, q4 * 4:(q4 + 1) * 4])
                    return xts

                # ---- PE warm-up: dummy matmuls on a memset tile keep the
                # HAM clock-gate open through the DMA startup dead zone.
                # The result feeds a tiny real output so DCE keeps it.
                wscr = sa.tile([128, 256], BF16, tag="wscr")
                nc.vector.memset(wscr[:], 0.125)
                warm_ps = psA1.tile([128, 256], F32, tag="t_aux", name="warm_ps")
                for wi in range(16):
                    nc.tensor.matmul(warm_ps[:], wscr[:, 0:128], wscr[:],
                                     start=(wi == 0), stop=(wi == 15))
                wsum = sa.tile([1, 8], F32, tag="wsum")
                nc.scalar.activation(wsum[:], warm_ps[0:1, 0:8], ACTF.Copy)

                # ---- startup DMA, sync queue, in first-use order ----
                xts_pre = xp.tile([128, NW * TCH], BF16, tag="xts", name="xts0")
                xv0 = xts_pre[:].rearrange("p (c t) -> p c t", t=TCH)
                nc.sync.dma_start(out=wk_v[:, 0:8], in_=wk4[:, 0:8])
                nc.sync.dma_start(out=xv0[:, 0:4], in_=xt4[:, 0, 0:4])
                nc.sync.dma_start(out=xv0[:, 4:8], in_=xt4[:, 0, 4:8])
                nc.sync.dma_start(out=wk_v[:, 8:16], in_=wk4[:, 8:16])
                nc.sync.dma_start(out=xv0[:, 8:12], in_=xt4[:, 0, 8:12])
                nc.sync.dma_start(out=xv0[:, 12:16], in_=xt4[:, 0, 12:16])
                for q4 in range(4):
                    nc.sync.dma_start(out=wq_v[:, q4 * 4:(q4 + 1) * 4],
                                      in_=wq4[:, q4 * 4:(q4 + 1) * 4])
                nc.sync.dma_start(out=wv_v[:, 0:8], in_=wv4[:, 0:8])
                nc.sync.dma_start(out=wv_v[:, 8:16], in_=wv4[:, 8:16])
                if shared_tables:
                    tabt_full = {}
                    for name in tab_names:
                        tt = tp.tile([128, T], BF16, tag=name, name=f"tab_{name}")
                        nc.sync.dma_start(out=tt[:], in_=tabs[name][:])
                        tabt_full[name] = tt
                wout_t = [woutp.tile([128, T], BF16, tag=f"wo{c}", name=f"wo{c}")
                          for c in range(4)]
                wout_r = wout.rearrange("(c p) t -> c p t", p=128)
                for c in range(4):
                    nc.sync.dma_start(out=wout_t[c][:], in_=wout_r[c])
                encT = [encp.tile([128, T], BF16, tag=f"encT{c}", name=f"encT{c}")
                        for c in range(4)]
                maskt = encp.tile([128, max(n_pat, 1) * 128], BF16,
                                  tag="maskt", name="maskt")
                nc.sync.dma_start(out=maskt[:], in_=masks_d[:])

                if shared_tables:
                    q_tabs = k_tabs = ("ct", "st", "ct", "st")
                else:
                    q_tabs = ("cq1", "sq1", "cq2", "sq2")
                    k_tabs = ("ck1", "sk1", "ck2", "sk2")
                units = [
                    (wk_t, 256, 0, k_tabs, kT, 0, None),
                    (wq_t, 512, 0, q_tabs, qT, 0, 0),
                    (wq_t, 512, 256, q_tabs, qT, 2, 1),
                ]

                # ---------- stage A helpers (pipelined norm tail) ----------
                def emit_unit_mains(ui, tci, xts, unit):
                    """Main projection matmuls + ACT squares.  Returns state
                    for the deferred tail."""
                    w_t, wcols, cbase, tkeys, dest, dbase, qhead = unit
                    ps1 = psA.tile([128, TCH], F32, tag="t_s0",
                                   name=f"ps1_{ui}_{tci}")
                    ps2 = psA.tile([128, TCH], F32, tag="t_s1",
                                   name=f"ps2_{ui}_{tci}")
                    # interleave the two output row-chunks per x quarter so
                    # accumulation starts as soon as each quarter lands
                    for q4 in range(4):
                        for ps, cc in ((ps1, 0), (ps2, 1)):
                            coff = cbase + cc * 128
                            for wc in range(4 * q4, 4 * q4 + 4):
                                nc.tensor.matmul(
                                    ps[:],
                                    w_t[:, wc * wcols + coff: wc * wcols + coff + 128],
                                    xts[:, wc * TCH:(wc + 1) * TCH],
                                    start=(wc == 0), stop=(wc == NW - 1),
                                )
                    par = ui % 2
                    sq1 = sa.tile([128, TCH], F32R, tag=f"sq1_{par}")
                    sq2 = sa.tile([128, TCH], F32R, tag=f"sq2_{par}")
                    nc.scalar.activation(sq1[:], ps1[:], ACTF.Square)
                    nc.scalar.activation(sq2[:], ps2[:], ACTF.Square)
                    return (ui, tci, unit, ps1, ps2, sq1, sq2)

                def emit_tail1(st):
                    """Variance matmuls (PE) + std sqrt (ACT)."""
                    ui, tci, unit, ps1, ps2, sq1, sq2 = st
                    qhead = unit[6]
                    par = ui % 2
                    psvar = psA1.tile([1, TCH], F32, tag="t_aux",
                                      name=f"pv_{ui}_{tci}")
                    nc.tensor.matmul(psvar[:], onesc[:], sq1[:], start=True, stop=False)
                    nc.tensor.matmul(psvar[:], onesc[:], sq2[:], start=False, stop=True)
                    if qhead is None:
                        stdv = sa.tile([1, TCH], F32R, tag=f"stdv_{par}")
                        nc.scalar.activation(stdv[:], psvar[:], ACTF.Sqrt,
                                             scale=1.0 / HEAD_DIM, bias=epsb[:])
                        return st + (stdv,)
                    stdvf = sa.tile([1, TCH], F32, tag=f"stdvf_{par}")
                    nc.scalar.activation(stdvf[:], psvar[:], ACTF.Sqrt,
                                         bias=epsbq[:])
                    return st + (stdvf,)

                def emit_tail2(st2, tabt):
                    """rstd broadcast / transpose (PE) + RoPE (DVE)."""
                    ui, tci, unit, ps1, ps2, sq1, sq2, stdx = st2
                    w_t, wcols, cbase, tkeys, dest, dbase, qhead = unit
                    t0 = tci * TCH
                    par = ui % 2
                    C1, S1, C2, S2 = (
                        (tabt[k][:, t0:t0 + TCH] if shared_tables
                         else tabt[k][:]) for k in tkeys)
                    m1 = sa.tile([128, TCH], F32, tag=f"m1_{par}")
                    m2 = sa.tile([128, TCH], F32, tag=f"m2_{par}")
                    m3 = sa.tile([128, TCH], F32, tag=f"m1_{par}", name=f"m3_{ui}_{tci}")
                    m4 = sa.tile([128, TCH], F32, tag=f"m2_{par}", name=f"m4_{ui}_{tci}")
                    if qhead is None:
                        # k: apply rstd via PE broadcast, fused into rope
                        psb = psA1.tile([128, TCH], F32, tag="t_aux",
                                        name=f"psb_{ui}_{tci}")
                        nc.tensor.matmul(psb[:], ones1[:], stdx[:],
                                         start=True, stop=True)
                        rb = sa.tile([128, TCH], F32, tag=f"rb_{par}")
                        nc.vector.reciprocal_approx_fast(out=rb[:], in_=psb[:])
                        a1 = sa.tile([128, TCH], F32, tag=f"a1_{par}")
                        a2 = sa.tile([128, TCH], F32, tag=f"a2_{par}")
                        nc.vector.tensor_tensor(a1[:], ps1[:], rb[:], AluOpType.mult)
                        nc.vector.tensor_tensor(a2[:], ps2[:], rb[:], AluOpType.mult)
                    else:
                        # q: defer 1/std to the stage-B logits scale;
                        # transpose 16*std per 128-tile via K=1 matmuls
                        sq_ps = psA1.tile([128, TCH // 128], F32, tag="t_aux",
                                          name=f"sqp_{ui}_{tci}")
                        for s in range(TCH // 128):
                            nc.tensor.matmul(
                                sq_ps[:, s:s + 1],
                                stdx[:, s * 128:(s + 1) * 128],
                                ones_f[:], start=True, stop=True)
                        stdq = sa.tile([128, TCH // 128], F32, tag=f"stdq_{par}")
                        nc.scalar.activation(stdq[:], sq_ps[:], ACTF.Copy)
                        nc.vector.reciprocal_approx_fast(
                            out=rstdq_c[qhead][:, tci * (TCH // 128):
                                               (tci + 1) * (TCH // 128)],
                            in_=stdq[:])
                        a1, a2 = ps1, ps2
                    nc.vector.tensor_tensor(m1[:], a1[:], C1, AluOpType.mult)
                    nc.vector.tensor_tensor(m2[:], a2[:], S2, AluOpType.mult)
                    nc.vector.tensor_tensor(
                        dest[dbase][:, t0:t0 + TCH], m1[:], m2[:], AluOpType.subtract)
                    nc.vector.tensor_tensor(m3[:], a2[:], C2, AluOpType.mult)
                    nc.vector.tensor_tensor(m4[:], a1[:], S1, AluOpType.mult)
                    nc.vector.tensor_tensor(
                        dest[dbase + 1][:, t0:t0 + TCH], m3[:], m4[:], AluOpType.add)

                def emit_v(tci, xts):
                    t0 = tci * TCH
                    vT_sb = sa.tile([128, 2 * TCH], BF16, tag="vTsb")
                    for cc in range(2):
                        psv = psA.tile([128, TCH], F32, tag="t_eps",
                                       name=f"psv_{tci}_{cc}")
                        for wc in range(NW):
                            nc.tensor.matmul(
                                psv[:],
                                wv_t[:, wc * 256 + cc * 128: wc * 256 + (cc + 1) * 128],
                                xts[:, wc * TCH:(wc + 1) * TCH],
                                start=(wc == 0), stop=(wc == NW - 1),
                            )
                        nc.scalar.activation(vT_sb[:, cc * TCH:(cc + 1) * TCH],
                                             psv[:], ACTF.Copy)
                    for s in range(TCH // 128):
                        jc = tci * (TCH // 128) + s
                        for cc in range(2):
                            psvt = psA1.tile([128, 128], BF16, tag="t_aux",
                                             name=f"pvt_{jc}_{cc}")
                            nc.tensor.transpose(
                                psvt[:],
                                vT_sb[:, cc * TCH + s * 128: cc * TCH + (s + 1) * 128],
                                ident[:])
                            nc.vector.tensor_copy(
                                vt[:, jc * 256 + cc * 128: jc * 256 + (cc + 1) * 128],
                                psvt[:])

                # ---------- stage B: per-tile attention ----------
                pend = [None]

                def emit_pv_tile(it, L, pts):
                    nl = len(L)
                    for cc in range(2):
                        eps = psE.tile([128, 256], F32, tag="t_eps",
                                       name=f"eps{it}_{cc}")
                        for i, jc in enumerate(L):
                            nc.tensor.matmul(
                                eps[:],
                                vt[:, jc * 256 + cc * 128: jc * 256 + (cc + 1) * 128],
                                pts[:, i * 256:(i + 1) * 256],
                                start=(i == 0), stop=(i == nl - 1),
                            )
                        for h in range(2):
                            nc.vector.tensor_copy(
                                encT[2 * h + cc][:, it * 128:(it + 1) * 128],
                                eps[:, h * 128:(h + 1) * 128])

                def emit_out_tile(tt, fine=False):
                    ob = outp.tile([128, T], F16, tag="ob", name=f"ob{tt}")
                    for nb in range(4):
                        ops = psE.tile([128, 512], F32, tag="t_eps",
                                       name=f"ops{tt}_{nb}")
                        for cc in range(4):
                            nc.tensor.matmul(
                                ops[:],
                                encT[cc][:, tt * 128:(tt + 1) * 128],
                                wout_t[cc][:, nb * 512:(nb + 1) * 512],
                                start=(cc == 0), stop=(cc == 3),
                            )
                        if nb % 2 == 0:
                            nc.scalar.activation(
                                ob[:, nb * 512:(nb + 1) * 512], ops[:], ACTF.Copy)
                        else:
                            nc.vector.tensor_copy(
                                ob[:, nb * 512:(nb + 1) * 512], ops[:])
                        if fine:
                            nc.sync.dma_start(
                                out=yp[tt * 128:(tt + 1) * 128,
                                       nb * 512:(nb + 1) * 512],
                                in_=ob[:, nb * 512:(nb + 1) * 512])
                    if not fine:
                        nc.sync.dma_start(
                            out=yp[tt * 128:(tt + 1) * 128, :], in_=ob[:])

                def flush_pend(fine=False):
                    if pend[0] is not None:
                        emit_pv_tile(*pend[0])
                        emit_out_tile(pend[0][0], fine=fine)
                        pend[0] = None

                def emit_attn_tile(it):
                    jst = js[it]
                    base = jst // 128
                    row = plan[it]
                    runs = _runs(row)
                    L = tile_chunks[it]
                    nl = len(L)
                    pts = ptp.tile([128, nl * 256], BF16, tag="pts",
                                   name=f"pts{it}")
                    den2 = sbp.tile([128, 2], F32, tag="den2", name=f"den2_{it}")
                    pdict = {}
                    for h in range(2):
                        plist = []
                        for c0, c1 in runs:
                            col = c0 * 128
                            for pw in _pieces((c1 - c0) * 128):
                                ps = psS.tile(
                                    [128, pw], F32,
                                    tag=f"t_s{len(plist) % 2}",
                                    name=f"S{it}_{h}_{len(plist)}")
                                for cc in range(2):
                                    nc.tensor.matmul(
                                        ps[:],
                                        qT[2 * h + cc][:, it * 128:(it + 1) * 128],
                                        kT[cc][:, jst + col: jst + col + pw],
                                        start=(cc == 0), stop=(cc == 1),
                                    )
                                plist.append((ps, col, pw))
                                col += pw
                        for ps, col, pw in plist:
                            for i in range(col // 128, (col + pw) // 128):
                                cls, slot = row[i]
                                if cls == "mask":
                                    off = i * 128 - col
                                    nc.vector.tensor_tensor(
                                        ps[:, off:off + 128],
                                        ps[:, off:off + 128],
                                        maskt[:, slot * 128:(slot + 1) * 128],
                                        AluOpType.add)
                        P_t = sbp.tile([128, Wb], BF16, tag=f"P{h}",
                                       name=f"P{it}_{h}")
                        dslot = den2[:, h:h + 1]
                        dparts = sbp.tile([128, 4], F32, tag=f"dp{h}",
                                          name=f"dp{it}_{h}")
                        for pi, (ps, col, pw) in enumerate(plist):
                            nc.scalar.activation(
                                P_t[:, col:col + pw], ps[:], ACTF.Exp,
                                scale=rstdq_c[h][:, it:it + 1],
                                accum_out=(dslot if len(plist) == 1
                                           else dparts[:, pi:pi + 1]))
                        if len(plist) > 1:
                            nc.gpsimd.tensor_tensor(
                                dslot, dparts[:, 0:1], dparts[:, 1:2],
                                AluOpType.add)
                            for pi in range(2, len(plist)):
                                nc.gpsimd.tensor_tensor(
                                    dslot, dslot, dparts[:, pi:pi + 1],
                                    AluOpType.add)
                        pdict[h] = (P_t, runs)
                    rden = sbp.tile([128, 2], F32, tag="rden", name=f"rden{it}")
                    nc.vector.reciprocal_approx_fast(out=rden[:], in_=den2[:])
                    Pns = []
                    for h in range(2):
                        P_t, truns = pdict[h]
                        Pn = sbp.tile([128, Wb], BF16, tag=f"Pn{h}",
                                      name=f"Pn{it}_{h}")
                        for c0, c1 in truns:
                            nc.vector.tensor_scalar_mul(
                                Pn[:, c0 * 128:c1 * 128],
                                P_t[:, c0 * 128:c1 * 128], rden[:, h:h + 1])
                        Pns.append(Pn)
                    # interleave the previous tile's PV + out-projection here:
                    # the PE chews on it while ACT/DVE run this tile's exp,
                    # normalize and the copies feeding the transposes below.
                    flush_pend()
                    pts_v = pts[:].rearrange("p (i f c) -> p i f c", f=2, c=128)
                    for h in range(2):
                        P_t, truns = pdict[h]
                        Pn = Pns[h]
                        for c0, c1 in truns:
                            idx0 = L.index(base + c0)
                            lj = c0
                            while lj < c1:
                                nb = min(3, c1 - lj)
                                ps_t = psT.tile([128, 3 * 128], BF16, tag="t_aux",
                                                name=f"ptps{it}_{h}_{lj}")
                                for k in range(nb):
                                    nc.tensor.transpose(
                                        ps_t[:, k * 128:(k + 1) * 128],
                                        Pn[:, (lj + k) * 128:(lj + k + 1) * 128],
                                        ident[:])
                                nc.vector.tensor_copy(
                                    pts_v[:, idx0 + lj - c0: idx0 + lj - c0 + nb,
                                          h, :],
                                    ps_t[:, 0:nb * 128].rearrange(
                                        "p (k c) -> p k c", c=128))
                                lj += nb
                    pend[0] = (it, L, pts)

                # ---------------- main schedule ----------------
                xts_next = xts_pre
                for tci in range(NTCH):
                    xts = xts_next
                    xts_next = load_xts(tci + 1) if tci + 1 < NTCH else None
                    if shared_tables:
                        tabt = tabt_full
                    else:
                        t0 = tci * TCH
                        tabt = {}
                        for name in dict.fromkeys(q_tabs + k_tabs):
                            tt = tp.tile([128, TCH], BF16, tag=name,
                                         name=f"tab_{name}_{tci}")
                            nc.gpsimd.dma_start(out=tt[:],
                                                in_=tabs[name][:, t0:t0 + TCH])
                            tabt[name] = tt
                        # non-shared tables are per-tci slices of the full ones
                        tabt = {k: v for k, v in tabt.items()}
                    atiles = [] if tci == 0 else list(range(4 * (tci - 1),
                                                           4 * (tci - 1) + 4))
                    st_k = emit_unit_mains(0, tci, xts, units[0])
                    st_q1 = emit_unit_mains(1, tci, xts, units[1])
                    st1_k = emit_tail1(st_k)
                    emit_v(tci, xts)
                    emit_tail2(st1_k, tabt)
                    st_q2 = emit_unit_mains(2, tci, xts, units[2])
                    st1_q1 = emit_tail1(st_q1)
                    if atiles:
                        emit_attn_tile(atiles[0])
                    emit_tail2(st1_q1, tabt)
                    if atiles:
                        emit_attn_tile(atiles[1])
                    st1_q2 = emit_tail1(st_q2)
                    if atiles:
                        emit_attn_tile(atiles[2])
                    emit_tail2(st1_q2, tabt)
                    if atiles:
                        emit_attn_tile(atiles[3])
                for it in range(4 * (NTCH - 1), NT):
                    emit_attn_tile(it)
                flush_pend(fine=True)
                nc.sync.dma_start(out=warm_out[:], in_=wsum[:])
            ps_shared.__exit__(None, None, None)

    nc.compile()
    return nc


def kernel(x, positions, attn_mask, wq, wkv, wout, q_scale, k_scale):
    BF = ml_dtypes.bfloat16
    x = np.ascontiguousarray(x, np.float32)
    positions = np.asarray(positions)
    wq = np.ascontiguousarray(wq, np.float32)
    wkv = np.ascontiguousarray(wkv, np.float32)
    wout = np.ascontiguousarray(wout, np.float32)
    q_scale = np.asarray(q_scale, np.float32)
    k_scale = np.asarray(k_scale, np.float32)

    valid, Wb, js = _geometry(positions, attn_mask)
    shared = not (q_scale.any() or k_scale.any())
    plan, patterns = _classify(valid, js, Wb)
    n_pat = len(patterns)

    key = (Wb, js, plan, n_pat, shared)
    if key not in _prog_cache:
        _prog_cache[key] = _build(Wb, js, plan, n_pat, shared)
    nc = _prog_cache[key]

    # per-batch additive mask tiles: 0 where valid, -3e4 elsewhere,
    # packed [128, n_pat*128] (partition = query row)
    masks = np.zeros((B, max(n_pat, 1), 128, 128), np.float32)
    for p, pat in enumerate(patterns):
        masks[:, p] = np.where(pat, 0.0, -3.0e4)
    masks = masks.astype(ml_dtypes.bfloat16)
    masks_packed = [np.ascontiguousarray(
        masks[b].transpose(1, 0, 2).reshape(128, max(n_pat, 1) * 128))
        for b in range(B)]

    ident = np.eye(128, dtype=BF)
    ones1 = np.ones((1, 128), np.float32)
    onesc = np.ones((128, 1), np.float32)

    def tile_pcm(w, m):
        # [WIDTH, m] -> [128, NW, m] partition-major
        return np.ascontiguousarray(
            w.reshape(NW, 128, m).transpose(1, 0, 2).astype(BF))

    in_maps = []
    for core in range(8):
        b, kh = divmod(core, NUM_KV_HEADS)
        xT = x[b].T  # [WIDTH, T]
        xt4 = np.ascontiguousarray(
            xT.reshape(NW, 128, NTCH, TCH).transpose(1, 2, 0, 3).astype(BF))
        m = {
            "xt4": xt4,
            "wq4": tile_pcm(wq[:, kh * 512:(kh + 1) * 512], 512),
            "wk4": tile_pcm(wkv[:, kh * 256:(kh + 1) * 256], 256),
            "wv4": tile_pcm(wkv[:, 1024 + kh * 256: 1024 + (kh + 1) * 256], 256),
            "wout": np.ascontiguousarray(wout[kh * 512:(kh + 1) * 512, :].astype(BF)),
            "ident": ident, "ones1": ones1, "onesc": onesc,
            "masks": masks_packed[b],
        }
        if shared:
            ct, st, _, _ = _rope_tables(positions[b], np.zeros(HEAD_DIM, np.float32))
            m["ct"], m["st"] = ct.astype(BF), st.astype(BF)
        else:
            for nm, tb in zip(("cq1", "sq1", "cq2", "sq2"),
                              _rope_tables(positions[b], q_scale)):
                m[nm] = tb.astype(BF)
            for nm, tb in zip(("ck1", "sk1", "ck2", "sk2"),
                              _rope_tables(positions[b], k_scale)):
                m[nm] = tb.astype(BF)
        in_maps.append(m)

    res = run_bass_kernel_spmd(nc, in_maps, list(range(8)))
    kernel._last_results = res
    out = np.empty((B, T, T), np.float32)
    for b in range(B):
        acc = res.results[b * NUM_KV_HEADS]["yp"].astype(np.float32)
        for kh in range(1, NUM_KV_HEADS):
            acc += res.results[b * NUM_KV_HEADS + kh]["yp"].astype(np.float32)
        out[b] = acc
    return out
